# revision 5
# baseline (speedup 1.0000x reference)
"""GraphUNet on 8 Trainium2 NeuronCores — 4-launch SPMD design.

Device launches (host does only top-k / gathers / degree formulas / tiny GEMMs):
  A: GCN0            — An0 @ z0 aggregation, transposed form, fp8 adjacency
  B: MM0 + GCN1      — C1^T column-blocks (fp8, exact: A entries small ints)
                       fused with the level-1 GCN aggregation
  C: MM1 + GCN2      — same at level 1, bf16
  D: MM2 + GCN3 + up — level-2 pool matmul, GCN3, and the full up path
                       (3 levels) with 3 small AllGather collectives

Up-path identity (avoids device-side scatter):
  gcn(A, res + scatter(xu)) = (An @ res + An[:, keep] @ xu) @ W + b
Host passes normalized An^T / (An[:,keep])^T column-slices; the device runs
aggregation matmuls into psum h^T[16, own] plus a tiny [16,128]x[16,16]
W-multiply per 128-chunk.

Numerics: adjacency exact in fp8 (level 0) / bf16 (levels 1-2), z-streams
bf16, psum fp32, normalization fp32 on host. ~2e-3 rel err vs reference.
"""
import os
import numpy as np
import ml_dtypes

import concourse.mybir as mybir
import concourse.tile as tile
from concourse import bacc
from concourse.bass_utils import run_bass_kernel_spmd

N, E, F, D = 4096, 131072, 14, 16
NC = 8
KS = [2048, 1024, 512]

bf16 = ml_dtypes.bfloat16
fp8 = ml_dtypes.float8_e4m3

MOCK = os.environ.get("BASS_MOCK") == "1"
TRACE = os.environ.get("BASS_PROF") == "1"
_tns = [0]
_hw_ok = [True]
_neffs = {}

_FP8_INT_LUT = np.arange(33, dtype=np.float32).astype(fp8).view(np.uint8)


def _int_to_fp8(a_int):
    """Exact fp8 cast for small-int arrays via LUT."""
    return _FP8_INT_LUT[np.ascontiguousarray(a_int)].view(fp8)


def _bf(a):
    return np.ascontiguousarray(a).astype(bf16)


def _chunked(a, nch):
    return np.ascontiguousarray(np.asarray(a).reshape(nch, 128, -1))


# ================================================================ NEFF builders

def build_gcn0():
    """out[16, 512] = relu(disb * (z^T @ A0effT_own) + b)."""
    if "A" in _neffs:
        return _neffs["A"]
    Wg = N // NC
    CH = N // 128
    nc = bacc.Bacc("TRN2", target_bir_lowering=False, debug=False, num_devices=NC)
    at = nc.dram_tensor("at", [CH, 128, Wg], mybir.dt.float8e4, kind="ExternalInput")
    z = nc.dram_tensor("z", [CH, 128, D], mybir.dt.bfloat16, kind="ExternalInput")
    disb = nc.dram_tensor("disb", [D, Wg], mybir.dt.float32, kind="ExternalInput")
    bias = nc.dram_tensor("bias", [D, 1], mybir.dt.float32, kind="ExternalInput")
    out = nc.dram_tensor("out", [D, Wg], mybir.dt.float32, kind="ExternalOutput")

    with tile.TileContext(nc) as tc:
        with (
            tc.tile_pool(name="sb", bufs=1) as sb,
            tc.tile_pool(name="ps", bufs=1, space="PSUM") as ps,
        ):
            att = sb.tile([128, CH, Wg], mybir.dt.float8e4)
            zt = sb.tile([128, CH, D], mybir.dt.bfloat16)
            for c in range(CH):
                nc.sync.dma_start(att[:, c, :], at[c])
                nc.sync.dma_start(zt[:, c, :], z[c])
            db = sb.tile([D, Wg], mybir.dt.float32)
            nc.sync.dma_start(db[:], disb[:])
            bi = sb.tile([D, 1], mybir.dt.float32)
            nc.sync.dma_start(bi[:], bias[:])

            pu = ps.tile([D, Wg], mybir.dt.float32, space="PSUM")
            for c in range(CH):
                nc.tensor.matmul(pu[:], lhsT=zt[:, c, :], rhs=att[:, c, :],
                                 start=(c == 0), stop=(c == CH - 1))
            t = sb.tile([D, Wg], mybir.dt.float32)
            nc.vector.tensor_mul(t[:], pu[:], db[:])
            o = sb.tile([D, Wg], mybir.dt.float32)
            nc.scalar.activation(o[:], t[:], mybir.ActivationFunctionType.Relu,
                                 bias=bi[:, 0:1], scale=1.0)
            nc.sync.dma_start(out[:], o[:])
    nc.finalize()
    _neffs["A"] = nc
    return nc


def build_mmgcn(lvl):
    """Levels 0/1: C^T column-block matmul + fused pooled-level GCN."""
    key = f"L{lvl}"
    if key in _neffs:
        return _neffs[key]
    n = [N, KS[0]][lvl]
    k = KS[lvl]
    wb = k // NC
    CH = n // 128
    MB = k // 128
    XB = wb // 128
    dt_in = mybir.dt.float8e4 if lvl == 0 else mybir.dt.bfloat16

    nc = bacc.Bacc("TRN2", target_bir_lowering=False, debug=False, num_devices=NC)
    lh = nc.dram_tensor("lh", [CH, 128, k], dt_in, kind="ExternalInput")
    rh = nc.dram_tensor("rh", [CH, 128, wb], dt_in, kind="ExternalInput")
    zin = nc.dram_tensor("zin", [MB, 128, D], mybir.dt.bfloat16, kind="ExternalInput")
    mm = nc.dram_tensor("mm", [MB, 128, wb], mybir.dt.bfloat16, kind="ExternalInput")
    ma = nc.dram_tensor("ma", [MB, 128, wb], mybir.dt.bfloat16, kind="ExternalInput")
    dis = nc.dram_tensor("dis", [128, XB], mybir.dt.float32, kind="ExternalInput")
    biasT = nc.dram_tensor("biasT", [128, D], mybir.dt.float32, kind="ExternalInput")
    ct = nc.dram_tensor("ct", [MB, 128, wb], mybir.dt.bfloat16, kind="ExternalOutput")
    xo = nc.dram_tensor("xo", [XB, 128, D], mybir.dt.float32, kind="ExternalOutput")

    with tile.TileContext(nc) as tc:
        with (
            tc.tile_pool(name="sb", bufs=1) as sb,
            tc.tile_pool(name="st", bufs=3) as st,
            tc.tile_pool(name="ps", bufs=3, space="PSUM") as ps,
            tc.tile_pool(name="px", bufs=2, space="PSUM") as px,
        ):
            lt = sb.tile([128, CH, k], dt_in)
            rt = sb.tile([128, CH, wb], dt_in)
            for c in range(CH):
                nc.sync.dma_start(lt[:, c, :], lh[c])
                nc.sync.dma_start(rt[:, c, :], rh[c])
            zt = sb.tile([128, MB, D], mybir.dt.bfloat16)
            mmt = sb.tile([128, MB, wb], mybir.dt.bfloat16)
            mat = sb.tile([128, MB, wb], mybir.dt.bfloat16)
            for m in range(MB):
                nc.sync.dma_start(zt[:, m, :], zin[m])
                nc.sync.dma_start(mmt[:, m, :], mm[m])
                nc.sync.dma_start(mat[:, m, :], ma[m])
            dst = sb.tile([128, XB], mybir.dt.float32)
            nc.sync.dma_start(dst[:], dis[:])
            bt = sb.tile([128, D], mybir.dt.float32)
            nc.sync.dma_start(bt[:], biasT[:])

            c1t = sb.tile([128, MB, wb], mybir.dt.bfloat16)
            for m in range(MB):
                pc = ps.tile([128, wb], mybir.dt.float32, space="PSUM", tag="pc")
                for c in range(CH):
                    nc.tensor.matmul(pc[:], lhsT=lt[:, c, m * 128:(m + 1) * 128],
                                     rhs=rt[:, c, :],
                                     start=(c == 0), stop=(c == CH - 1))
                cb = st.tile([128, wb], mybir.dt.bfloat16, tag="cb")
                nc.vector.tensor_mul(cb[:], pc[:], mmt[:, m, :])
                nc.vector.tensor_add(c1t[:, m, :], cb[:], mat[:, m, :])
                nc.sync.dma_start(ct[m], c1t[:, m, :])

            for xb in range(XB):
                pg = px.tile([128, D], mybir.dt.float32, space="PSUM", tag="pg")
                for m in range(MB):
                    nc.tensor.matmul(pg[:],
                                     lhsT=c1t[:, m, xb * 128:(xb + 1) * 128],
                                     rhs=zt[:, m, :],
                                     start=(m == 0), stop=(m == MB - 1))
                t1 = st.tile([128, D], mybir.dt.float32, tag="t1")
                nc.scalar.activation(t1[:], pg[:],
                                     mybir.ActivationFunctionType.Copy,
                                     bias=0.0, scale=dst[:, xb:xb + 1])
                nc.vector.tensor_add(t1[:], t1[:], bt[:])
                xt = st.tile([128, D], mybir.dt.float32, tag="xt")
                nc.vector.tensor_scalar_max(xt[:], t1[:], 0.0)
                nc.sync.dma_start(xo[xb], xt[:])
    nc.finalize()
    _neffs[key] = nc
    return nc


def _allgather(nc, tc, src_tile, rows, bounce_in, bounce_out, dst_tile, nchunk):
    """AllGather src_tile ([rows<=128,16] or [128,nb,16]) into dst[128,nchunk,16]."""
    nb = (rows + 127) // 128
    with tc.tile_critical():
        sem = nc.alloc_semaphore(f"ag_{nc.next_id()}")
        if rows <= 128:
            nc.sync.dma_start(bounce_in[:, :], src_tile[:rows, :]).then_inc(sem, 16)
        else:
            for q in range(nb):
                nc.sync.dma_start(bounce_in[q * 128:(q + 1) * 128, :],
                                  src_tile[:, q, :]).then_inc(sem, 16)
        nc.gpsimd.wait_ge(sem, 16 * nb)
        nc.gpsimd.collective_compute(
            "AllGather", mybir.AluOpType.bypass,
            replica_groups=[list(range(NC))],
            ins=[bounce_in.ap().opt()],
            outs=[bounce_out.ap().opt()],
        ).then_inc(sem, 1)
        nc.sync.wait_ge(sem, 16 * nb + 1)
        for c in range(nchunk):
            nc.sync.dma_start(dst_tile[:, c, :],
                              bounce_out[c * 128:(c + 1) * 128, :]).then_inc(sem, 16)
        nc.sync.wait_ge(sem, 16 * (nb + nchunk) + 1)


def build_final():
    """Launch D: MM2 + GCN3 + full up path."""
    if "D" in _neffs:
        return _neffs["D"]
    n2, k3, wb3 = KS[1], KS[2], KS[2] // NC   # 1024, 512, 64
    CH2, MB3 = n2 // 128, k3 // 128           # 8, 4
    W0, W1, W2 = 1024 // NC, 2048 // NC, N // NC  # 128, 256, 512
    dt = mybir.dt.bfloat16
    f32 = mybir.dt.float32

    nc = bacc.Bacc("TRN2", target_bir_lowering=False, debug=False, num_devices=NC)
    lh = nc.dram_tensor("lh", [CH2, 128, k3], dt, kind="ExternalInput")
    rh = nc.dram_tensor("rh", [CH2, 128, wb3], dt, kind="ExternalInput")
    z3 = nc.dram_tensor("z3", [MB3, 128, D], dt, kind="ExternalInput")
    mm3 = nc.dram_tensor("mm3", [MB3, 128, wb3], dt, kind="ExternalInput")
    ma3 = nc.dram_tensor("ma3", [MB3, 128, wb3], dt, kind="ExternalInput")
    dis3 = nc.dram_tensor("dis3", [128, 1], f32, kind="ExternalInput")
    b3T = nc.dram_tensor("b3T", [128, D], f32, kind="ExternalInput")
    a2t = nc.dram_tensor("a2t", [CH2, 128, W0], dt, kind="ExternalInput")
    m2t = nc.dram_tensor("m2t", [MB3, 128, W0], dt, kind="ExternalInput")
    x2n = nc.dram_tensor("x2n", [CH2, 128, D], dt, kind="ExternalInput")
    wu0 = nc.dram_tensor("wu0", [D, D], dt, kind="ExternalInput")
    bu0T = nc.dram_tensor("bu0T", [128, D], f32, kind="ExternalInput")
    a1t = nc.dram_tensor("a1t", [16, 128, W1], dt, kind="ExternalInput")
    m1t = nc.dram_tensor("m1t", [8, 128, W1], dt, kind="ExternalInput")
    x1n = nc.dram_tensor("x1n", [16, 128, D], dt, kind="ExternalInput")
    wu1 = nc.dram_tensor("wu1", [D, D], dt, kind="ExternalInput")
    bu1T = nc.dram_tensor("bu1T", [128, D], f32, kind="ExternalInput")
    a0t = nc.dram_tensor("a0t", [32, 128, W2], dt, kind="ExternalInput")
    m0t = nc.dram_tensor("m0t", [16, 128, W2], dt, kind="ExternalInput")
    x0n = nc.dram_tensor("x0n", [32, 128, D], dt, kind="ExternalInput")
    wu2 = nc.dram_tensor("wu2", [D, D], dt, kind="ExternalInput")
    bu2T = nc.dram_tensor("bu2T", [128, D], f32, kind="ExternalInput")
    xfin = nc.dram_tensor("xfin", [4, 128, D], f32, kind="ExternalOutput")

    g3i = nc.dram_tensor("g3i", [wb3, D], dt)
    g3o = nc.dram_tensor("g3o", [k3, D], dt)
    g0i = nc.dram_tensor("g0i", [128, D], dt)
    g0o = nc.dram_tensor("g0o", [1024, D], dt)
    g1i = nc.dram_tensor("g1i", [256, D], dt)
    g1o = nc.dram_tensor("g1o", [2048, D], dt)

    with tile.TileContext(nc) as tc:
        with (
            tc.tile_pool(name="sb", bufs=1) as sb,
            tc.tile_pool(name="st", bufs=4) as st,
            tc.tile_pool(name="ps", bufs=3, space="PSUM") as ps,
            tc.tile_pool(name="px", bufs=2, space="PSUM") as px,
        ):
            def load(t, dram, nchunk):
                for c in range(nchunk):
                    nc.sync.dma_start(t[:, c, :], dram[c])

            lt = sb.tile([128, CH2, k3], dt); load(lt, lh, CH2)
            rt = sb.tile([128, CH2, wb3], dt); load(rt, rh, CH2)
            z3t = sb.tile([128, MB3, D], dt); load(z3t, z3, MB3)
            mm3t = sb.tile([128, MB3, wb3], dt); load(mm3t, mm3, MB3)
            ma3t = sb.tile([128, MB3, wb3], dt); load(ma3t, ma3, MB3)
            a2tt = sb.tile([128, CH2, W0], dt); load(a2tt, a2t, CH2)
            m2tt = sb.tile([128, MB3, W0], dt); load(m2tt, m2t, MB3)
            x2t = sb.tile([128, CH2, D], dt); load(x2t, x2n, CH2)
            a1tt = sb.tile([128, 16, W1], dt); load(a1tt, a1t, 16)
            m1tt = sb.tile([128, 8, W1], dt); load(m1tt, m1t, 8)
            x1t = sb.tile([128, 16, D], dt); load(x1t, x1n, 16)
            a0tt = sb.tile([128, 32, W2], dt); load(a0tt, a0t, 32)
            m0tt = sb.tile([128, 16, W2], dt); load(m0tt, m0t, 16)
            x0t = sb.tile([128, 32, D], dt); load(x0t, x0n, 32)
            wu0t = sb.tile([D, D], dt); nc.sync.dma_start(wu0t[:], wu0[:])
            wu1t = sb.tile([D, D], dt); nc.sync.dma_start(wu1t[:], wu1[:])
            wu2t = sb.tile([D, D], dt); nc.sync.dma_start(wu2t[:], wu2[:])
            d3t = sb.tile([128, 1], f32); nc.sync.dma_start(d3t[:], dis3[:])
            b3t = sb.tile([128, D], f32); nc.sync.dma_start(b3t[:], b3T[:])
            bu0t = sb.tile([128, D], f32); nc.sync.dma_start(bu0t[:], bu0T[:])
            bu1t = sb.tile([128, D], f32); nc.sync.dma_start(bu1t[:], bu1T[:])
            bu2t = sb.tile([128, D], f32); nc.sync.dma_start(bu2t[:], bu2T[:])

            # ---- MM2: C3^T column-blocks
            c3t = sb.tile([128, MB3, wb3], dt)
            for m in range(MB3):
                pc = ps.tile([128, wb3], f32, space="PSUM", tag="pc")
                for c in range(CH2):
                    nc.tensor.matmul(pc[:], lhsT=lt[:, c, m * 128:(m + 1) * 128],
                                     rhs=rt[:, c, :],
                                     start=(c == 0), stop=(c == CH2 - 1))
                cb = st.tile([128, wb3], dt, tag="cb")
                nc.vector.tensor_mul(cb[:], pc[:], mm3t[:, m, :])
                nc.vector.tensor_add(c3t[:, m, :], cb[:], ma3t[:, m, :])

            # ---- GCN3 -> x3 node-major [64, 16]
            pg = px.tile([128, D], f32, space="PSUM", tag="pq")
            for m in range(MB3):
                nc.tensor.matmul(pg[:wb3, :], lhsT=c3t[:, m, :], rhs=z3t[:, m, :],
                                 start=(m == 0), stop=(m == MB3 - 1))
            t1 = st.tile([128, D], f32, tag="t1")
            nc.scalar.activation(t1[:wb3, :], pg[:wb3, :],
                                 mybir.ActivationFunctionType.Copy,
                                 bias=0.0, scale=d3t[:wb3, 0:1])
            nc.vector.tensor_add(t1[:wb3, :], t1[:wb3, :], b3t[:wb3, :])
            x3sb = st.tile([128, D], dt, tag="x3sb")
            nc.vector.tensor_scalar_max(x3sb[:wb3, :], t1[:wb3, :], 0.0)

            x3g = sb.tile([128, MB3, D], dt)
            _allgather(nc, tc, x3sb, wb3, g3i, g3o, x3g, MB3)

            # ---- u0
            ph = px.tile([16, W2], f32, space="PSUM", tag="ph")
            for c in range(CH2):
                nc.tensor.matmul(ph[:, :W0], lhsT=x2t[:, c, :], rhs=a2tt[:, c, :],
                                 start=(c == 0), stop=False)
            for m in range(MB3):
                nc.tensor.matmul(ph[:, :W0], lhsT=x3g[:, m, :], rhs=m2tt[:, m, :],
                                 start=False, stop=(m == MB3 - 1))
            h0 = st.tile([16, W0], dt, tag="h0")
            nc.vector.tensor_copy(h0[:], ph[:, :W0])
            pq = px.tile([128, D], f32, space="PSUM", tag="pq")
            nc.tensor.matmul(pq[:], lhsT=h0[:, :], rhs=wu0t[:], start=True, stop=True)
            t2 = st.tile([128, D], f32, tag="t2")
            nc.vector.tensor_add(t2[:], pq[:], bu0t[:])
            xu0 = st.tile([128, D], dt, tag="xu0")
            nc.vector.tensor_scalar_max(xu0[:], t2[:], 0.0)

            xu0g = sb.tile([128, 8, D], dt)
            _allgather(nc, tc, xu0, 128, g0i, g0o, xu0g, 8)

            # ---- u1
            ph1 = px.tile([16, W2], f32, space="PSUM", tag="ph")
            for c in range(16):
                nc.tensor.matmul(ph1[:, :W1], lhsT=x1t[:, c, :], rhs=a1tt[:, c, :],
                                 start=(c == 0), stop=False)
            for m in range(8):
                nc.tensor.matmul(ph1[:, :W1], lhsT=xu0g[:, m, :], rhs=m1tt[:, m, :],
                                 start=False, stop=(m == 7))
            h1 = st.tile([16, W1], dt, tag="h0")
            nc.vector.tensor_copy(h1[:], ph1[:, :W1])
            xu1 = st.tile([128, 2, D], dt, tag="xu1")
            for q in range(2):
                pq1 = px.tile([128, D], f32, space="PSUM", tag="pq")
                nc.tensor.matmul(pq1[:], lhsT=h1[:, q * 128:(q + 1) * 128],
                                 rhs=wu1t[:], start=True, stop=True)
                t3 = st.tile([128, D], f32, tag="t2")
                nc.vector.tensor_add(t3[:], pq1[:], bu1t[:])
                nc.vector.tensor_scalar_max(xu1[:, q, :], t3[:], 0.0)

            xu1g = sb.tile([128, 16, D], dt)
            _allgather(nc, tc, xu1, 256, g1i, g1o, xu1g, 16)

            # ---- u2 (final, no relu)
            ph2 = px.tile([16, W2], f32, space="PSUM", tag="ph")
            for c in range(32):
                nc.tensor.matmul(ph2[:], lhsT=x0t[:, c, :], rhs=a0tt[:, c, :],
                                 start=(c == 0), stop=False)
            for m in range(16):
                nc.tensor.matmul(ph2[:], lhsT=xu1g[:, m, :], rhs=m0tt[:, m, :],
                                 start=False, stop=(m == 15))
            h2 = st.tile([16, W2], dt, tag="h2")
            nc.vector.tensor_copy(h2[:], ph2[:])
            for q in range(4):
                pq2 = px.tile([128, D], f32, space="PSUM", tag="pq")
                nc.tensor.matmul(pq2[:], lhsT=h2[:, q * 128:(q + 1) * 128],
                                 rhs=wu2t[:], start=True, stop=True)
                t4 = st.tile([128, D], f32, tag="t2")
                nc.vector.tensor_add(t4[:], pq2[:], bu2t[:])
                nc.sync.dma_start(xfin[q], t4[:])
    nc.finalize()
    _neffs["D"] = nc
    return nc


# ================================================================ launch helper

def _run(nc, in_maps):
    if MOCK or not _hw_ok[0]:
        return None
    try:
        r = run_bass_kernel_spmd(nc, in_maps, core_ids=list(range(NC)), trace=TRACE)
    except Exception as e:
        import sys
        print(f"DEVICE LAUNCH FAILED ({type(e).__name__}: {e}); host fallback",
              file=sys.stderr)
        _hw_ok[0] = False
        return None
    if getattr(r, "exec_time_ns", None):
        _tns[0] += r.exec_time_ns
    return r.results


# ================================================================ host pipeline

def _diag_masks(k, wb, c):
    """Masks zeroing/setting the C_eff^T diagonal inside core c's column block."""
    mmask = np.ones((k, wb), np.float32)
    amask = np.zeros((k, wb), np.float32)
    j = np.arange(wb)
    mmask[c * wb + j, j] = 0.0
    amask[c * wb + j, j] = 1.0
    return mmask, amask


def kernel(**inputs):
    x = np.asarray(inputs["x"], np.float32)
    ei = np.asarray(inputs["edge_index"]).astype(np.int64)
    W = {k: np.asarray(v, np.float32) for k, v in inputs.items()
         if k not in ("x", "edge_index")}

    # ---- adjacency (integer, exact)
    Ai = np.bincount((ei[0] * N + ei[1]).ravel(), minlength=N * N)\
        .reshape(N, N).astype(np.int16)
    d0 = np.diagonal(Ai).copy()
    fix0 = (d0 == 0).astype(np.int16)
    A0eff = Ai.copy(); np.fill_diagonal(A0eff, d0 + fix0)
    Ab = Ai.copy(); np.fill_diagonal(Ab, 1)

    deg0 = A0eff.sum(1, dtype=np.int64).astype(np.float32)
    dis0 = 1.0 / np.sqrt(deg0)

    # ---- launch A: GCN0
    z0 = _bf(dis0[:, None] * (x @ W["W_d0"]))
    A0effT = np.ascontiguousarray(A0eff.T)
    ncA = build_gcn0()
    Wg = N // NC
    maps = []
    for c in range(NC):
        sl = slice(c * Wg, (c + 1) * Wg)
        maps.append({
            "at": _chunked(_int_to_fp8(A0effT[:, sl]), 32),
            "z": _chunked(z0, 32),
            "disb": np.ascontiguousarray(
                np.broadcast_to(dis0[sl][None, :], (D, Wg))).astype(np.float32),
            "bias": W["b_d0"].reshape(D, 1).astype(np.float32),
        })
    outs = _run(ncA, maps)
    if outs is None:
        agg = A0eff.astype(np.float32) @ z0.astype(np.float32)
        x0 = np.maximum(dis0[:, None] * agg + W["b_d0"], 0.0)
    else:
        x0 = np.concatenate([o["out"] for o in outs], axis=1).T.copy()

    # ---- levels 0 and 1 on device (launches B, C)
    xs = [x0]
    keeps, diss, CeffTs = [], [dis0], []
    Ab_f = Ab.astype(np.float32)      # current level Ab (fp32, for deg formulas)
    AbT_bf = None                     # current C_eff^T (bf16) for levels >= 1
    xcur = x0
    for lvl in range(3):
        n = [N, KS[0], KS[1]][lvl]
        k = KS[lvl]
        wb = k // NC
        p = W[f"p{lvl}"]
        s = np.tanh((xcur @ p) / np.linalg.norm(p))
        order = np.argsort(-s, kind="stable")
        keep = np.sort(order[:k])
        vals = s[keep]
        keeps.append(keep)

        # pooled-level degrees from current Ab (closed form, host)
        v = Ab_f[:, keep].sum(1)
        w_ = Ab_f @ v
        diagC = np.einsum('ij,ji->i', Ab_f[keep, :], Ab_f[:, keep])
        deg = w_[keep] - diagC + 1.0
        dis = 1.0 / np.sqrt(deg)
        diss.append(dis)

        zlv = _bf(dis[:, None] * ((xcur[keep] * vals[:, None]) @ W[f"W_d{lvl+1}"]))

        if lvl == 2:
            # handled inside launch D
            z3_bf, dis3_v, keep2 = zlv, dis, keep
            break

        if lvl == 0:
            lh_full = _int_to_fp8(np.ascontiguousarray(Ab[:, keep]))
            AbT = np.ascontiguousarray(Ab.T)
            rh_of = lambda rows: _int_to_fp8(np.ascontiguousarray(AbT[:, rows]))
        else:
            lh_full = _bf(AbT_bf[keep, :].T)
            rh_of = lambda rows: np.ascontiguousarray(AbT_bf[:, rows])

        MB, XB = k // 128, wb // 128
        ncL = build_mmgcn(lvl)
        maps = []
        for c in range(NC):
            rows = keep[c * wb:(c + 1) * wb]
            mmask, amask = _diag_masks(k, wb, c)
            maps.append({
                "lh": _chunked(lh_full, n // 128),
                "rh": _chunked(rh_of(rows), n // 128),
                "zin": _chunked(zlv, MB),
                "mm": _chunked(_bf(mmask), MB),
                "ma": _chunked(_bf(amask), MB),
                "dis": np.ascontiguousarray(
                    dis[c * wb:(c + 1) * wb].reshape(XB, 128).T).astype(np.float32),
                "biasT": np.broadcast_to(
                    W[f"b_d{lvl+1}"][None, :], (128, D)).astype(np.float32).copy(),
            })
        outs = _run(ncL, maps)
        if outs is None:
            C = Ab_f[keep, :] @ Ab_f[:, keep]
            Ceff = C.copy(); np.fill_diagonal(Ceff, 1.0)
            CeffT = _bf(Ceff.T)
            aggf = CeffT.astype(np.float32).T @ zlv.astype(np.float32)
            xn = np.maximum(dis[:, None] * aggf + W[f"b_d{lvl+1}"], 0.0)
        else:
            CeffT = np.concatenate(
                [o["ct"].reshape(k, wb) for o in outs], axis=1)
            xn = np.concatenate(
                [o["xo"].reshape(XB * 128, D) for o in outs], axis=0)
        CeffTs.append(CeffT)
        xs.append(xn)
        xcur = xn
        AbT_bf = CeffT
        Ab_f = CeffT.T.astype(np.float32)

    x1, x2 = xs[1], xs[2]
    keep0, keep1 = keeps[0], keeps[1]
    dis1, dis2 = diss[1], diss[2]
    C1effT, C2effT = CeffTs[0], CeffTs[1]

    # ---- launch D inputs
    k3, wb3 = KS[2], KS[2] // NC
    # MM2 operands from C2_eff
    lh2 = _bf(C2effT[keep2, :].T)                       # Ab2[:, keep2] [1024, 512]
    # normalized up-path matrices (fp32 on host, bf16 slices per core)
    C1T_f = C1effT.astype(np.float32)
    C2T_f = C2effT.astype(np.float32)
    An1T = dis1[:, None] * C1T_f * dis1[None, :]        # [2048, 2048] = An1^T
    An2T = dis2[:, None] * C2T_f * dis2[None, :]        # [1024, 1024] = An_u0^T
    An0T = (dis0[:, None] * A0effT.astype(np.float32) * dis0[None, :])  # [4096,4096]
    An1T_bf = _bf(An1T); An2T_bf = _bf(An2T); An0T_bf = _bf(An0T)
    M1T_bf = _bf(An1T[keep1, :])                        # (An1[:,keep1])^T [1024, 2048]
    M2T_bf = _bf(An2T[keep2, :])                        # [512, 1024]
    M0T_bf = _bf(An0T[keep0, :])                        # [2048, 4096]

    x0n = _bf(x0); x1n = _bf(x1); x2n = _bf(x2)
    bt = lambda b: np.broadcast_to(b[None, :], (128, D)).astype(np.float32).copy()
    wu = lambda k_: _bf(W[k_])
    ncD = build_final()
    maps = []
    for c in range(NC):
        rows3 = slice(c * wb3, (c + 1) * wb3)
        mmask, amask = _diag_masks(k3, wb3, c)
        d3 = np.zeros((128, 1), np.float32)
        d3[:wb3, 0] = dis3_v[rows3]
        maps.append({
            "lh": _chunked(lh2, 8),
            "rh": _chunked(np.ascontiguousarray(C2effT[:, keep2[rows3]]), 8),
            "z3": _chunked(z3_bf, 4),
            "mm3": _chunked(_bf(mmask), 4),
            "ma3": _chunked(_bf(amask), 4),
            "dis3": d3,
            "b3T": bt(W["b_d3"]),
            "a2t": _chunked(np.ascontiguousarray(An2T_bf[:, c * 128:(c + 1) * 128]), 8),
            "m2t": _chunked(np.ascontiguousarray(M2T_bf[:, c * 128:(c + 1) * 128]), 4),
            "x2n": _chunked(x2n, 8),
            "wu0": wu("W_u0"), "bu0T": bt(W["b_u0"]),
            "a1t": _chunked(np.ascontiguousarray(An1T_bf[:, c * 256:(c + 1) * 256]), 16),
            "m1t": _chunked(np.ascontiguousarray(M1T_bf[:, c * 256:(c + 1) * 256]), 8),
            "x1n": _chunked(x1n, 16),
            "wu1": wu("W_u1"), "bu1T": bt(W["b_u1"]),
            "a0t": _chunked(np.ascontiguousarray(An0T_bf[:, c * 512:(c + 1) * 512]), 32),
            "m0t": _chunked(np.ascontiguousarray(M0T_bf[:, c * 512:(c + 1) * 512]), 16),
            "x0n": _chunked(x0n, 32),
            "wu2": wu("W_u2"), "bu2T": bt(W["b_u2"]),
        })
    outs = _run(ncD, maps)
    if outs is None:
        return _host_final(x0, x1, x2, keep0, keep1, keep2, dis3_v, z3_bf,
                           C2effT, lh2, An0T, An1T, An2T, W)
    return np.concatenate(
        [o["xfin"].reshape(4 * 128, D) for o in outs], axis=0).astype(np.float32)


def _host_final(x0, x1, x2, keep0, keep1, keep2, dis3, z3_bf, C2effT, lh2,
                An0T, An1T, An2T, W):
    """Numpy emulation of launch D (mock / fallback)."""
    Ab2 = C2effT.T.astype(np.float32)
    C3 = Ab2[keep2, :] @ Ab2[:, keep2]
    C3eff = C3.copy(); np.fill_diagonal(C3eff, 1.0)
    C3eff = _bf(C3eff).astype(np.float32)
    x3 = np.maximum(dis3[:, None] * (C3eff @ z3_bf.astype(np.float32))
                    + W["b_d3"], 0.0)
    h0 = An2T.T @ _bf(x2).astype(np.float32) + An2T.T[:, keep2] @ _bf(x3).astype(np.float32)
    xu0 = np.maximum(h0 @ W["W_u0"] + W["b_u0"], 0.0)
    h1 = An1T.T @ _bf(x1).astype(np.float32) + An1T.T[:, keep1] @ _bf(xu0).astype(np.float32)
    xu1 = np.maximum(h1 @ W["W_u1"] + W["b_u1"], 0.0)
    h2 = An0T.T @ _bf(x0).astype(np.float32) + An0T.T[:, keep0] @ _bf(xu1).astype(np.float32)
    return (h2 @ W["W_u2"] + W["b_u2"]).astype(np.float32)


# revision 12
# speedup vs baseline: 1.4928x; 1.4928x over previous
"""GraphUNet on 8 Trainium2 NeuronCores — 4-launch SPMD design.

Device launches (host does only top-k / gathers / degree formulas / tiny GEMMs):
  A: GCN0            — An0 @ z0 aggregation, transposed form, fp8 adjacency
  B: MM0 + GCN1      — C1^T column-blocks (fp8, exact: A entries small ints)
                       fused with the level-1 GCN aggregation
  C: MM1 + GCN2      — same at level 1, bf16
  D: MM2 + GCN3 + up — level-2 pool matmul, GCN3, and the full up path
                       (3 levels) with 3 small AllGather collectives

Up-path identity (avoids device-side scatter):
  gcn(A, res + scatter(xu)) = (An @ res + An[:, keep] @ xu) @ W + b
Host passes normalized An^T / (An[:,keep])^T column-slices; the device runs
aggregation matmuls into psum h^T[16, own] plus a tiny [16,128]x[16,16]
W-multiply per 128-chunk.

Numerics: adjacency exact in fp8 (level 0) / bf16 (levels 1-2), z-streams
bf16, psum fp32, normalization fp32 on host. ~2e-3 rel err vs reference.
"""
import os
import numpy as np
import ml_dtypes

import concourse.mybir as mybir
import concourse.tile as tile
from concourse import bacc
from concourse.bass_utils import run_bass_kernel_spmd

N, E, F, D = 4096, 131072, 14, 16
NC = 8
KS = [2048, 1024, 512]

bf16 = ml_dtypes.bfloat16
fp8 = ml_dtypes.float8_e4m3

MOCK = os.environ.get("BASS_MOCK") == "1"
TRACE = os.environ.get("BASS_PROF") == "1"
_tns = [0]
_launch_ns = []
_hw_ok = [True]
_neffs = {}

_FP8_INT_LUT = np.arange(33, dtype=np.float32).astype(fp8).view(np.uint8)


def _int_to_fp8(a_int):
    """Exact fp8 cast for small-int arrays via LUT."""
    return _FP8_INT_LUT[np.ascontiguousarray(a_int)].view(fp8)


def _bf(a):
    return np.ascontiguousarray(a).astype(bf16)


def _pm(a, nch):
    """[nch*128, w] row-major -> [128, nch, w] partition-major."""
    a = np.asarray(a)
    return np.ascontiguousarray(a.reshape(nch, 128, -1).transpose(1, 0, 2))


# ================================================================ NEFF builders

def build_gcn0():
    """out[16, 512] = relu(disb * (z^T @ A0effT_own) + b)."""
    if "A" in _neffs:
        return _neffs["A"]
    Wg = N // NC
    CH = N // 128
    nc = bacc.Bacc("TRN2", target_bir_lowering=False, debug=False, num_devices=NC)
    at = nc.dram_tensor("at", [128, CH, Wg], mybir.dt.float8e4, kind="ExternalInput")
    z = nc.dram_tensor("z", [128, CH, D], mybir.dt.bfloat16, kind="ExternalInput")
    disb = nc.dram_tensor("disb", [D, Wg], mybir.dt.float32, kind="ExternalInput")
    bias = nc.dram_tensor("bias", [D, 1], mybir.dt.float32, kind="ExternalInput")
    out = nc.dram_tensor("out", [D, Wg], mybir.dt.float32, kind="ExternalOutput")

    with tile.TileContext(nc) as tc:
        with (
            tc.tile_pool(name="sb", bufs=1) as sb,
            tc.tile_pool(name="ps", bufs=1, space="PSUM") as ps,
        ):
            att = sb.tile([128, CH, Wg], mybir.dt.float8e4)
            zt = sb.tile([128, CH, D], mybir.dt.bfloat16)
            nc.sync.dma_start(zt[:], z[:])
            G = 4
            for g in range(G):
                gs = slice(g * CH // G, (g + 1) * CH // G)
                nc.sync.dma_start(att[:, gs, :], at[:, gs, :])
            db = sb.tile([D, Wg], mybir.dt.float32)
            nc.sync.dma_start(db[:], disb[:])
            bi = sb.tile([D, 1], mybir.dt.float32)
            nc.sync.dma_start(bi[:], bias[:])

            pu = ps.tile([D, Wg], mybir.dt.float32, space="PSUM")
            for c in range(CH):
                nc.tensor.matmul(pu[:], lhsT=zt[:, c, :], rhs=att[:, c, :],
                                 start=(c == 0), stop=(c == CH - 1))
            t = sb.tile([D, Wg], mybir.dt.float32)
            nc.vector.tensor_mul(t[:], pu[:], db[:])
            o = sb.tile([D, Wg], mybir.dt.float32)
            nc.scalar.activation(o[:], t[:], mybir.ActivationFunctionType.Relu,
                                 bias=bi[:, 0:1], scale=1.0)
            nc.sync.dma_start(out[:], o[:])
    nc.finalize()
    _neffs["A"] = nc
    return nc


def build_mmgcn(lvl):
    """Levels 0/1: C^T column-block matmul + fused pooled-level GCN."""
    key = f"L{lvl}"
    if key in _neffs:
        return _neffs[key]
    n = [N, KS[0]][lvl]
    k = KS[lvl]
    wb = k // NC
    CH = n // 128
    MB = k // 128
    XB = wb // 128
    dt_in = mybir.dt.float8e4 if lvl == 0 else mybir.dt.bfloat16

    nc = bacc.Bacc("TRN2", target_bir_lowering=False, debug=False, num_devices=NC)
    lh = nc.dram_tensor("lh", [128, CH, k], dt_in, kind="ExternalInput")
    rh = nc.dram_tensor("rh", [128, CH, wb], dt_in, kind="ExternalInput")
    zin = nc.dram_tensor("zin", [128, MB, D], mybir.dt.bfloat16, kind="ExternalInput")
    mm = nc.dram_tensor("mm", [128, MB, wb], mybir.dt.bfloat16, kind="ExternalInput")
    ma = nc.dram_tensor("ma", [128, MB, wb], mybir.dt.bfloat16, kind="ExternalInput")
    dis = nc.dram_tensor("dis", [128, XB], mybir.dt.float32, kind="ExternalInput")
    biasT = nc.dram_tensor("biasT", [128, D], mybir.dt.float32, kind="ExternalInput")
    ct = nc.dram_tensor("ct", [128, MB, wb], mybir.dt.bfloat16, kind="ExternalOutput")
    xo = nc.dram_tensor("xo", [XB, 128, D], mybir.dt.float32, kind="ExternalOutput")

    with tile.TileContext(nc) as tc:
        with (
            tc.tile_pool(name="sb", bufs=1) as sb,
            tc.tile_pool(name="st", bufs=3) as st,
            tc.tile_pool(name="ps", bufs=3, space="PSUM") as ps,
            tc.tile_pool(name="px", bufs=2, space="PSUM") as px,
        ):
            lt = sb.tile([128, CH, k], dt_in)
            rt = sb.tile([128, CH, wb], dt_in)
            nc.sync.dma_start(rt[:], rh[:])
            G = 8
            for g in range(G):
                gs = slice(g * CH // G, (g + 1) * CH // G)
                nc.sync.dma_start(lt[:, gs, :], lh[:, gs, :])
            zt = sb.tile([128, MB, D], mybir.dt.bfloat16)
            mmt = sb.tile([128, MB, wb], mybir.dt.bfloat16)
            mat = sb.tile([128, MB, wb], mybir.dt.bfloat16)
            nc.sync.dma_start(zt[:], zin[:])
            nc.sync.dma_start(mmt[:], mm[:])
            nc.sync.dma_start(mat[:], ma[:])
            dst = sb.tile([128, XB], mybir.dt.float32)
            nc.sync.dma_start(dst[:], dis[:])
            bt = sb.tile([128, D], mybir.dt.float32)
            nc.sync.dma_start(bt[:], biasT[:])

            c1t = sb.tile([128, MB, wb], mybir.dt.bfloat16)
            for m in range(MB):
                pc = ps.tile([128, wb], mybir.dt.float32, space="PSUM", tag="pc")
                for c in range(CH):
                    nc.tensor.matmul(pc[:], lhsT=lt[:, c, m * 128:(m + 1) * 128],
                                     rhs=rt[:, c, :],
                                     start=(c == 0), stop=(c == CH - 1))
                cb = st.tile([128, wb], mybir.dt.bfloat16, tag="cb")
                nc.vector.tensor_mul(cb[:], pc[:], mmt[:, m, :])
                nc.vector.tensor_add(c1t[:, m, :], cb[:], mat[:, m, :])
                nc.sync.dma_start(ct[:, m, :], c1t[:, m, :])

            for xb in range(XB):
                pg = px.tile([128, D], mybir.dt.float32, space="PSUM", tag="pg")
                for m in range(MB):
                    nc.tensor.matmul(pg[:],
                                     lhsT=c1t[:, m, xb * 128:(xb + 1) * 128],
                                     rhs=zt[:, m, :],
                                     start=(m == 0), stop=(m == MB - 1))
                t1 = st.tile([128, D], mybir.dt.float32, tag="t1")
                nc.scalar.activation(t1[:], pg[:],
                                     mybir.ActivationFunctionType.Copy,
                                     bias=0.0, scale=dst[:, xb:xb + 1])
                nc.vector.tensor_add(t1[:], t1[:], bt[:])
                xt = st.tile([128, D], mybir.dt.float32, tag="xt")
                nc.vector.tensor_scalar_max(xt[:], t1[:], 0.0)
                nc.sync.dma_start(xo[xb], xt[:])
    nc.finalize()
    _neffs[key] = nc
    return nc


def _allgather(nc, tc, src_tile, rows, bounce_in, bounce_out, dst_tile, nchunk):
    """AllGather src_tile ([rows<=128,16] or [128,nb,16]) into dst[128,nchunk,16]."""
    nb = (rows + 127) // 128
    with tc.tile_critical():
        sem = nc.alloc_semaphore(f"ag_{nc.next_id()}")
        if rows <= 128:
            nc.sync.dma_start(bounce_in[:, :], src_tile[:rows, :]).then_inc(sem, 16)
        else:
            for q in range(nb):
                nc.sync.dma_start(bounce_in[q * 128:(q + 1) * 128, :],
                                  src_tile[:, q, :]).then_inc(sem, 16)
        nc.gpsimd.wait_ge(sem, 16 * nb)
        nc.gpsimd.collective_compute(
            "AllGather", mybir.AluOpType.bypass,
            replica_groups=[list(range(NC))],
            ins=[bounce_in.ap().opt()],
            outs=[bounce_out.ap().opt()],
        ).then_inc(sem, 1)
        nc.sync.wait_ge(sem, 16 * nb + 1)
        for c in range(nchunk):
            nc.sync.dma_start(dst_tile[:, c, :],
                              bounce_out[c * 128:(c + 1) * 128, :]).then_inc(sem, 16)
        nc.sync.wait_ge(sem, 16 * (nb + nchunk) + 1)


def build_final():
    """Launch D: MM2 + GCN3 + full up path."""
    if "D" in _neffs:
        return _neffs["D"]
    n2, k3, wb3 = KS[1], KS[2], KS[2] // NC   # 1024, 512, 64
    CH2, MB3 = n2 // 128, k3 // 128           # 8, 4
    W0, W1, W2 = 1024 // NC, 2048 // NC, N // NC  # 128, 256, 512
    dt = mybir.dt.bfloat16
    f32 = mybir.dt.float32

    nc = bacc.Bacc("TRN2", target_bir_lowering=False, debug=False, num_devices=NC)
    lh = nc.dram_tensor("lh", [128, CH2, k3], dt, kind="ExternalInput")
    rh = nc.dram_tensor("rh", [128, CH2, wb3], dt, kind="ExternalInput")
    z3 = nc.dram_tensor("z3", [128, MB3, D], dt, kind="ExternalInput")
    mm3 = nc.dram_tensor("mm3", [128, MB3, wb3], dt, kind="ExternalInput")
    ma3 = nc.dram_tensor("ma3", [128, MB3, wb3], dt, kind="ExternalInput")
    dis3 = nc.dram_tensor("dis3", [128, 1], f32, kind="ExternalInput")
    b3T = nc.dram_tensor("b3T", [128, D], f32, kind="ExternalInput")
    a2t = nc.dram_tensor("a2t", [128, CH2, W0], dt, kind="ExternalInput")
    m2t = nc.dram_tensor("m2t", [128, MB3, W0], dt, kind="ExternalInput")
    x2n = nc.dram_tensor("x2n", [128, CH2, D], dt, kind="ExternalInput")
    wu0 = nc.dram_tensor("wu0", [D, D], dt, kind="ExternalInput")
    bu0T = nc.dram_tensor("bu0T", [128, D], f32, kind="ExternalInput")
    a1t = nc.dram_tensor("a1t", [128, 16, W1], dt, kind="ExternalInput")
    m1t = nc.dram_tensor("m1t", [128, 8, W1], dt, kind="ExternalInput")
    x1n = nc.dram_tensor("x1n", [128, 16, D], dt, kind="ExternalInput")
    wu1 = nc.dram_tensor("wu1", [D, D], dt, kind="ExternalInput")
    bu1T = nc.dram_tensor("bu1T", [128, D], f32, kind="ExternalInput")
    a0t = nc.dram_tensor("a0t", [128, 32, W2], dt, kind="ExternalInput")
    m0t = nc.dram_tensor("m0t", [128, 16, W2], dt, kind="ExternalInput")
    x0n = nc.dram_tensor("x0n", [128, 32, D], dt, kind="ExternalInput")
    wu2 = nc.dram_tensor("wu2", [D, D], dt, kind="ExternalInput")
    bu2T = nc.dram_tensor("bu2T", [128, D], f32, kind="ExternalInput")
    xfin = nc.dram_tensor("xfin", [4, 128, D], f32, kind="ExternalOutput")

    g3i = nc.dram_tensor("g3i", [wb3, D], dt)
    g3o = nc.dram_tensor("g3o", [k3, D], dt)
    g0i = nc.dram_tensor("g0i", [128, D], dt)
    g0o = nc.dram_tensor("g0o", [1024, D], dt)
    g1i = nc.dram_tensor("g1i", [256, D], dt)
    g1o = nc.dram_tensor("g1o", [2048, D], dt)

    with tile.TileContext(nc) as tc:
        with (
            tc.tile_pool(name="sb", bufs=1) as sb,
            tc.tile_pool(name="st", bufs=4) as st,
            tc.tile_pool(name="ps", bufs=3, space="PSUM") as ps,
            tc.tile_pool(name="px", bufs=2, space="PSUM") as px,
        ):
            def load(t, dram):
                nc.sync.dma_start(t[:], dram[:])

            lt = sb.tile([128, CH2, k3], dt); load(lt, lh)
            rt = sb.tile([128, CH2, wb3], dt); load(rt, rh)
            z3t = sb.tile([128, MB3, D], dt); load(z3t, z3)
            mm3t = sb.tile([128, MB3, wb3], dt); load(mm3t, mm3)
            ma3t = sb.tile([128, MB3, wb3], dt); load(ma3t, ma3)
            a2tt = sb.tile([128, CH2, W0], dt); load(a2tt, a2t)
            m2tt = sb.tile([128, MB3, W0], dt); load(m2tt, m2t)
            x2t = sb.tile([128, CH2, D], dt); load(x2t, x2n)
            a1tt = sb.tile([128, 16, W1], dt); load(a1tt, a1t)
            m1tt = sb.tile([128, 8, W1], dt); load(m1tt, m1t)
            x1t = sb.tile([128, 16, D], dt); load(x1t, x1n)
            a0tt = sb.tile([128, 32, W2], dt)
            for g in range(4):
                nc.sync.dma_start(a0tt[:, g * 8:(g + 1) * 8, :],
                                  a0t[:, g * 8:(g + 1) * 8, :])
            m0tt = sb.tile([128, 16, W2], dt)
            for g in range(2):
                nc.sync.dma_start(m0tt[:, g * 8:(g + 1) * 8, :],
                                  m0t[:, g * 8:(g + 1) * 8, :])
            x0t = sb.tile([128, 32, D], dt); load(x0t, x0n)
            wu0t = sb.tile([D, D], dt); nc.sync.dma_start(wu0t[:], wu0[:])
            wu1t = sb.tile([D, D], dt); nc.sync.dma_start(wu1t[:], wu1[:])
            wu2t = sb.tile([D, D], dt); nc.sync.dma_start(wu2t[:], wu2[:])
            d3t = sb.tile([128, 1], f32); nc.sync.dma_start(d3t[:], dis3[:])
            b3t = sb.tile([128, D], f32); nc.sync.dma_start(b3t[:], b3T[:])
            bu0t = sb.tile([128, D], f32); nc.sync.dma_start(bu0t[:], bu0T[:])
            bu1t = sb.tile([128, D], f32); nc.sync.dma_start(bu1t[:], bu1T[:])
            bu2t = sb.tile([128, D], f32); nc.sync.dma_start(bu2t[:], bu2T[:])

            # ---- MM2: C3^T column-blocks
            c3t = sb.tile([128, MB3, wb3], dt)
            for m in range(MB3):
                pc = ps.tile([128, wb3], f32, space="PSUM", tag="pc")
                for c in range(CH2):
                    nc.tensor.matmul(pc[:], lhsT=lt[:, c, m * 128:(m + 1) * 128],
                                     rhs=rt[:, c, :],
                                     start=(c == 0), stop=(c == CH2 - 1))
                cb = st.tile([128, wb3], dt, tag="cb")
                nc.vector.tensor_mul(cb[:], pc[:], mm3t[:, m, :])
                nc.vector.tensor_add(c3t[:, m, :], cb[:], ma3t[:, m, :])

            # ---- GCN3 -> x3 node-major [64, 16]
            pg = px.tile([128, D], f32, space="PSUM", tag="pq")
            for m in range(MB3):
                nc.tensor.matmul(pg[:wb3, :], lhsT=c3t[:, m, :], rhs=z3t[:, m, :],
                                 start=(m == 0), stop=(m == MB3 - 1))
            t1 = st.tile([128, D], f32, tag="t1")
            nc.scalar.activation(t1[:wb3, :], pg[:wb3, :],
                                 mybir.ActivationFunctionType.Copy,
                                 bias=0.0, scale=d3t[:wb3, 0:1])
            nc.vector.tensor_add(t1[:wb3, :], t1[:wb3, :], b3t[:wb3, :])
            x3sb = st.tile([128, D], dt, tag="x3sb")
            nc.vector.tensor_scalar_max(x3sb[:wb3, :], t1[:wb3, :], 0.0)

            x3g = sb.tile([128, MB3, D], dt)
            _allgather(nc, tc, x3sb, wb3, g3i, g3o, x3g, MB3)

            # ---- u0
            ph = px.tile([16, W2], f32, space="PSUM", tag="ph")
            for c in range(CH2):
                nc.tensor.matmul(ph[:, :W0], lhsT=x2t[:, c, :], rhs=a2tt[:, c, :],
                                 start=(c == 0), stop=False)
            for m in range(MB3):
                nc.tensor.matmul(ph[:, :W0], lhsT=x3g[:, m, :], rhs=m2tt[:, m, :],
                                 start=False, stop=(m == MB3 - 1))
            h0 = st.tile([16, W0], dt, tag="h0")
            nc.vector.tensor_copy(h0[:], ph[:, :W0])
            pq = px.tile([128, D], f32, space="PSUM", tag="pq")
            nc.tensor.matmul(pq[:], lhsT=h0[:, :], rhs=wu0t[:], start=True, stop=True)
            t2 = st.tile([128, D], f32, tag="t2")
            nc.vector.tensor_add(t2[:], pq[:], bu0t[:])
            xu0 = st.tile([128, D], dt, tag="xu0")
            nc.vector.tensor_scalar_max(xu0[:], t2[:], 0.0)

            xu0g = sb.tile([128, 8, D], dt)
            _allgather(nc, tc, xu0, 128, g0i, g0o, xu0g, 8)

            # ---- u1
            ph1 = px.tile([16, W2], f32, space="PSUM", tag="ph")
            for c in range(16):
                nc.tensor.matmul(ph1[:, :W1], lhsT=x1t[:, c, :], rhs=a1tt[:, c, :],
                                 start=(c == 0), stop=False)
            for m in range(8):
                nc.tensor.matmul(ph1[:, :W1], lhsT=xu0g[:, m, :], rhs=m1tt[:, m, :],
                                 start=False, stop=(m == 7))
            h1 = st.tile([16, W1], dt, tag="h0")
            nc.vector.tensor_copy(h1[:], ph1[:, :W1])
            xu1 = st.tile([128, 2, D], dt, tag="xu1")
            for q in range(2):
                pq1 = px.tile([128, D], f32, space="PSUM", tag="pq")
                nc.tensor.matmul(pq1[:], lhsT=h1[:, q * 128:(q + 1) * 128],
                                 rhs=wu1t[:], start=True, stop=True)
                t3 = st.tile([128, D], f32, tag="t2")
                nc.vector.tensor_add(t3[:], pq1[:], bu1t[:])
                nc.vector.tensor_scalar_max(xu1[:, q, :], t3[:], 0.0)

            xu1g = sb.tile([128, 16, D], dt)
            _allgather(nc, tc, xu1, 256, g1i, g1o, xu1g, 16)

            # ---- u2 (final, no relu)
            ph2 = px.tile([16, W2], f32, space="PSUM", tag="ph")
            for c in range(32):
                nc.tensor.matmul(ph2[:], lhsT=x0t[:, c, :], rhs=a0tt[:, c, :],
                                 start=(c == 0), stop=False)
            for m in range(16):
                nc.tensor.matmul(ph2[:], lhsT=xu1g[:, m, :], rhs=m0tt[:, m, :],
                                 start=False, stop=(m == 15))
            h2 = st.tile([16, W2], dt, tag="h2")
            nc.vector.tensor_copy(h2[:], ph2[:])
            for q in range(4):
                pq2 = px.tile([128, D], f32, space="PSUM", tag="pq")
                nc.tensor.matmul(pq2[:], lhsT=h2[:, q * 128:(q + 1) * 128],
                                 rhs=wu2t[:], start=True, stop=True)
                t4 = st.tile([128, D], f32, tag="t2")
                nc.vector.tensor_add(t4[:], pq2[:], bu2t[:])
                nc.sync.dma_start(xfin[q], t4[:])
    nc.finalize()
    _neffs["D"] = nc
    return nc


# ================================================================ launch helper

def _run(nc, in_maps):
    if MOCK or not _hw_ok[0]:
        return None
    try:
        r = run_bass_kernel_spmd(nc, in_maps, core_ids=list(range(NC)), trace=TRACE)
    except Exception as e:
        import sys
        print(f"DEVICE LAUNCH FAILED ({type(e).__name__}: {e}); host fallback",
              file=sys.stderr)
        _hw_ok[0] = False
        return None
    if getattr(r, "exec_time_ns", None):
        _tns[0] += r.exec_time_ns
        _launch_ns.append(r.exec_time_ns)
    return r.results


# ================================================================ host pipeline

def _diag_masks(k, wb, c):
    """Masks zeroing/setting the C_eff^T diagonal inside core c's column block."""
    mmask = np.ones((k, wb), np.float32)
    amask = np.zeros((k, wb), np.float32)
    j = np.arange(wb)
    mmask[c * wb + j, j] = 0.0
    amask[c * wb + j, j] = 1.0
    return mmask, amask


def kernel(**inputs):
    x = np.asarray(inputs["x"], np.float32)
    ei = np.asarray(inputs["edge_index"]).astype(np.int64)
    W = {k: np.asarray(v, np.float32) for k, v in inputs.items()
         if k not in ("x", "edge_index")}

    # ---- adjacency (integer, exact)
    Ai = np.bincount((ei[0] * N + ei[1]).ravel(), minlength=N * N)\
        .reshape(N, N).astype(np.int16)
    d0 = np.diagonal(Ai).copy()
    fix0 = (d0 == 0).astype(np.int16)
    A0eff = Ai.copy(); np.fill_diagonal(A0eff, d0 + fix0)
    Ab = Ai.copy(); np.fill_diagonal(Ab, 1)

    deg0 = A0eff.sum(1, dtype=np.int64).astype(np.float32)
    dis0 = 1.0 / np.sqrt(deg0)

    # ---- launch A: GCN0
    z0 = _bf(dis0[:, None] * (x @ W["W_d0"]))
    A0effT = np.ascontiguousarray(A0eff.T)
    ncA = build_gcn0()
    Wg = N // NC
    maps = []
    for c in range(NC):
        sl = slice(c * Wg, (c + 1) * Wg)
        maps.append({
            "at": _pm(_int_to_fp8(A0effT[:, sl]), 32),
            "z": _pm(z0, 32),
            "disb": np.ascontiguousarray(
                np.broadcast_to(dis0[sl][None, :], (D, Wg))).astype(np.float32),
            "bias": W["b_d0"].reshape(D, 1).astype(np.float32),
        })
    outs = _run(ncA, maps)
    if outs is None:
        agg = A0eff.astype(np.float32) @ z0.astype(np.float32)
        x0 = np.maximum(dis0[:, None] * agg + W["b_d0"], 0.0)
    else:
        x0 = np.concatenate([o["out"] for o in outs], axis=1).T.copy()

    # ---- levels 0 and 1 on device (launches B, C)
    xs = [x0]
    keeps, diss, CeffTs = [], [dis0], []
    Ab_f = Ab.astype(np.float32)      # current level Ab (fp32, for deg formulas)
    AbT_bf = None                     # current C_eff^T (bf16) for levels >= 1
    xcur = x0
    for lvl in range(3):
        n = [N, KS[0], KS[1]][lvl]
        k = KS[lvl]
        wb = k // NC
        p = W[f"p{lvl}"]
        s = np.tanh((xcur @ p) / np.linalg.norm(p))
        order = np.argsort(-s, kind="stable")
        keep = np.sort(order[:k])
        vals = s[keep]
        keeps.append(keep)

        # pooled-level degrees from current Ab (closed form, host)
        v = Ab_f[:, keep].sum(1)
        w_ = Ab_f @ v
        diagC = np.einsum('ij,ji->i', Ab_f[keep, :], Ab_f[:, keep])
        deg = w_[keep] - diagC + 1.0
        dis = 1.0 / np.sqrt(deg)
        diss.append(dis)

        zlv = _bf(dis[:, None] * ((xcur[keep] * vals[:, None]) @ W[f"W_d{lvl+1}"]))

        if lvl == 2:
            # handled inside launch D
            z3_bf, dis3_v, keep2 = zlv, dis, keep
            break

        if lvl == 0:
            lh_full = _int_to_fp8(np.ascontiguousarray(Ab[:, keep]))
            AbT = np.ascontiguousarray(Ab.T)
            rh_of = lambda rows: _int_to_fp8(np.ascontiguousarray(AbT[:, rows]))
        else:
            lh_full = _bf(AbT_bf[keep, :].T)
            rh_of = lambda rows: np.ascontiguousarray(AbT_bf[:, rows])

        MB, XB = k // 128, wb // 128
        ncL = build_mmgcn(lvl)
        maps = []
        for c in range(NC):
            rows = keep[c * wb:(c + 1) * wb]
            mmask, amask = _diag_masks(k, wb, c)
            maps.append({
                "lh": _pm(lh_full, n // 128),
                "rh": _pm(rh_of(rows), n // 128),
                "zin": _pm(zlv, MB),
                "mm": _pm(_bf(mmask), MB),
                "ma": _pm(_bf(amask), MB),
                "dis": np.ascontiguousarray(
                    dis[c * wb:(c + 1) * wb].reshape(XB, 128).T).astype(np.float32),
                "biasT": np.broadcast_to(
                    W[f"b_d{lvl+1}"][None, :], (128, D)).astype(np.float32).copy(),
            })
        outs = _run(ncL, maps)
        if outs is None:
            C = Ab_f[keep, :] @ Ab_f[:, keep]
            Ceff = C.copy(); np.fill_diagonal(Ceff, 1.0)
            CeffT = _bf(Ceff.T)
            aggf = CeffT.astype(np.float32).T @ zlv.astype(np.float32)
            xn = np.maximum(dis[:, None] * aggf + W[f"b_d{lvl+1}"], 0.0)
        else:
            CeffT = np.concatenate(
                [np.asarray(o["ct"]).transpose(1, 0, 2).reshape(k, wb)
                 for o in outs], axis=1)
            xn = np.concatenate(
                [o["xo"].reshape(XB * 128, D) for o in outs], axis=0)
        CeffTs.append(CeffT)
        xs.append(xn)
        xcur = xn
        AbT_bf = CeffT
        Ab_f = CeffT.T.astype(np.float32)

    x1, x2 = xs[1], xs[2]
    keep0, keep1 = keeps[0], keeps[1]
    dis1, dis2 = diss[1], diss[2]
    C1effT, C2effT = CeffTs[0], CeffTs[1]

    # ---- launch D inputs
    k3, wb3 = KS[2], KS[2] // NC
    # MM2 operands from C2_eff
    lh2 = _bf(C2effT[keep2, :].T)                       # Ab2[:, keep2] [1024, 512]
    # normalized up-path matrices (fp32 on host, bf16 slices per core)
    C1T_f = C1effT.astype(np.float32)
    C2T_f = C2effT.astype(np.float32)
    An1T = dis1[:, None] * C1T_f * dis1[None, :]        # [2048, 2048] = An1^T
    An2T = dis2[:, None] * C2T_f * dis2[None, :]        # [1024, 1024] = An_u0^T
    An0T = (dis0[:, None] * A0effT.astype(np.float32) * dis0[None, :])  # [4096,4096]
    An1T_bf = _bf(An1T); An2T_bf = _bf(An2T); An0T_bf = _bf(An0T)
    M1T_bf = _bf(An1T[keep1, :])                        # (An1[:,keep1])^T [1024, 2048]
    M2T_bf = _bf(An2T[keep2, :])                        # [512, 1024]
    M0T_bf = _bf(An0T[keep0, :])                        # [2048, 4096]

    x0n = _bf(x0); x1n = _bf(x1); x2n = _bf(x2)
    bt = lambda b: np.broadcast_to(b[None, :], (128, D)).astype(np.float32).copy()
    wu = lambda k_: _bf(W[k_])
    ncD = build_final()
    maps = []
    for c in range(NC):
        rows3 = slice(c * wb3, (c + 1) * wb3)
        mmask, amask = _diag_masks(k3, wb3, c)
        d3 = np.zeros((128, 1), np.float32)
        d3[:wb3, 0] = dis3_v[rows3]
        maps.append({
            "lh": _pm(lh2, 8),
            "rh": _pm(np.ascontiguousarray(C2effT[:, keep2[rows3]]), 8),
            "z3": _pm(z3_bf, 4),
            "mm3": _pm(_bf(mmask), 4),
            "ma3": _pm(_bf(amask), 4),
            "dis3": d3,
            "b3T": bt(W["b_d3"]),
            "a2t": _pm(np.ascontiguousarray(An2T_bf[:, c * 128:(c + 1) * 128]), 8),
            "m2t": _pm(np.ascontiguousarray(M2T_bf[:, c * 128:(c + 1) * 128]), 4),
            "x2n": _pm(x2n, 8),
            "wu0": wu("W_u0"), "bu0T": bt(W["b_u0"]),
            "a1t": _pm(np.ascontiguousarray(An1T_bf[:, c * 256:(c + 1) * 256]), 16),
            "m1t": _pm(np.ascontiguousarray(M1T_bf[:, c * 256:(c + 1) * 256]), 8),
            "x1n": _pm(x1n, 16),
            "wu1": wu("W_u1"), "bu1T": bt(W["b_u1"]),
            "a0t": _pm(np.ascontiguousarray(An0T_bf[:, c * 512:(c + 1) * 512]), 32),
            "m0t": _pm(np.ascontiguousarray(M0T_bf[:, c * 512:(c + 1) * 512]), 16),
            "x0n": _pm(x0n, 32),
            "wu2": wu("W_u2"), "bu2T": bt(W["b_u2"]),
        })
    outs = _run(ncD, maps)
    if outs is None:
        return _host_final(x0, x1, x2, keep0, keep1, keep2, dis3_v, z3_bf,
                           C2effT, lh2, An0T, An1T, An2T, W)
    return np.concatenate(
        [o["xfin"].reshape(4 * 128, D) for o in outs], axis=0).astype(np.float32)


def _host_final(x0, x1, x2, keep0, keep1, keep2, dis3, z3_bf, C2effT, lh2,
                An0T, An1T, An2T, W):
    """Numpy emulation of launch D (mock / fallback)."""
    Ab2 = C2effT.T.astype(np.float32)
    C3 = Ab2[keep2, :] @ Ab2[:, keep2]
    C3eff = C3.copy(); np.fill_diagonal(C3eff, 1.0)
    C3eff = _bf(C3eff).astype(np.float32)
    x3 = np.maximum(dis3[:, None] * (C3eff @ z3_bf.astype(np.float32))
                    + W["b_d3"], 0.0)
    h0 = An2T.T @ _bf(x2).astype(np.float32) + An2T.T[:, keep2] @ _bf(x3).astype(np.float32)
    xu0 = np.maximum(h0 @ W["W_u0"] + W["b_u0"], 0.0)
    h1 = An1T.T @ _bf(x1).astype(np.float32) + An1T.T[:, keep1] @ _bf(xu0).astype(np.float32)
    xu1 = np.maximum(h1 @ W["W_u1"] + W["b_u1"], 0.0)
    h2 = An0T.T @ _bf(x0).astype(np.float32) + An0T.T[:, keep0] @ _bf(xu1).astype(np.float32)
    return (h2 @ W["W_u2"] + W["b_u2"]).astype(np.float32)
